# revision 4
# baseline (speedup 1.0000x reference)
"""Trainium2 Bass kernel for nn_MDNSeqModel: LSTM + encoder recurrence with
MDN decoder, data-parallel over batch across 8 NeuronCores.

Layout: feature-major activations [features(partitions), batch(free)],
batch 1024 sharded 8 ways -> 128 batch columns per core (= free dim of every
recurrent matmul). All matmul operands bf16, PSUM accumulation fp32,
elementwise fp32. Sigmoid computed as 0.5 + 0.5*tanh(x/2) (0.5 pre-folded
into the i/f/o weight rows) so tanh/exp/relu share one ACT table set.
State carried as cD = 2c and h2 = 2h so the 0.5 factors fold into
scalar_tensor_tensor ops and the weights that consume h.
"""
import os

import numpy as np
import ml_dtypes

STATE, ACT, Z, H = 21, 8, 128, 512
B, T = 1024, 128
NCORES = 8
BL = B // NCORES          # batch per core (free dim)
LOG_SQRT_2PI = 0.9189385332046727

bf16 = ml_dtypes.bfloat16

_CACHE = {}


def _split_multi_waits(nc, max_waits=1):
    """This walrus build rejects instructions carrying more than one sync-wait
    command; Tile's semaphore pass emits up to ~4 per instruction. Hoist the
    extras onto single-wait NOPs inserted just before, on the same engine
    (each engine executes its own stream in program order, so the semantics
    are identical)."""
    import concourse.mybir as mybir

    n_nops = 0
    for f in nc.m.functions:
        for bb in f.blocks:
            insts = bb.instructions
            out = []
            changed = False
            for ins in insts:
                si = ins.sync_info
                waits = list(si.on_wait) if si is not None else []
                if len(waits) > max_waits:
                    changed = True
                    extra = waits[:-max_waits]
                    for k, w in enumerate(extra):
                        nop = mybir.InstNoOp(
                            name=f"{ins.name}-wsplit{k}", engine=ins.engine)
                        nop.sync_info = mybir.SyncInfo(
                            on_update=[], on_wait=[w])
                        out.append(nop)
                        n_nops += 1
                    while len(si.on_wait) > max_waits:
                        si.on_wait.pop(0)
                out.append(ins)
            if changed:
                bb.instructions = out
    return n_nops


def _build_nc(t_steps):
    """Build the Bass module (same NEFF for all cores; SPMD over in_maps)."""
    import concourse.bass as bass
    import concourse.mybir as mybir
    import concourse.tile as tile

    dt = mybir.dt
    AF = mybir.ActivationFunctionType
    OP = mybir.AluOpType
    NT = t_steps * BL       # decoder free length

    nc = bass.Bass()
    P = nc.declare_dram_parameter

    # ---- inputs (per-core, host-prepped) ----
    wz_d = P("wz", [Z, 4 * H], dt.bfloat16, isOutput=False)        # W_ih z-part ^T
    wa_d = P("wa", [ACT + 1, 4 * H], dt.bfloat16, isOutput=False)  # [W_ih a-part | b]^T
    wh_d = P("wh", [4, 128, 4 * H], dt.bfloat16, isOutput=False)   # (0.5*W_hh)^T k-chunks
    w1_d = P("w1", [4, 128, 256], dt.bfloat16, isOutput=False)     # (0.5*enc_w1)^T
    b1_d = P("b1", [128, 2], dt.float32, isOutput=False)
    w2_d = P("w2", [2, 128, 128], dt.bfloat16, isOutput=False)     # enc_w2^T
    b2_d = P("b2", [128, 1], dt.float32, isOutput=False)
    wzz_d = P("wzz", [128, 2 * Z], dt.bfloat16, isOutput=False)    # enc_wz^T
    bzlo_d = P("bzlo", [128, 1], dt.float32, isOutput=False)
    bzhi_d = P("bzhi", [128, 1], dt.float32, isOutput=False)       # 1 + bz_hi
    dw1z_d = P("dw1z", [Z, 64], dt.bfloat16, isOutput=False)
    dw1o_d = P("dw1o", [STATE + 1, 64], dt.bfloat16, isOutput=False)  # [w1_obs | b1]^T
    dw2_d = P("dw2", [64, 64], dt.bfloat16, isOutput=False)
    db2_d = P("db2", [64, 1], dt.float32, isOutput=False)
    dw3_d = P("dw3", [64, 32], dt.bfloat16, isOutput=False)
    db3_d = P("db3", [32, 1], dt.float32, isOutput=False)
    dw4_d = P("dw4", [32, 16], dt.bfloat16, isOutput=False)
    db4_d = P("db4", [16, 1], dt.float32, isOutput=False)
    mw_d = P("mw", [16, STATE], dt.bfloat16, isOutput=False)
    mb_d = P("mb", [STATE, 1], dt.float32, isOutput=False)
    sw_d = P("sw", [16, STATE], dt.bfloat16, isOutput=False)
    sb_d = P("sb", [STATE, 1], dt.float32, isOutput=False)         # 1 + sig_b
    a_d = P("a_aug", [ACT + 1, NT], dt.bfloat16, isOutput=False)   # [a_t^T; 1]
    obs_d = P("obs_rep", [STATE + 1, 512], dt.bfloat16, isOutput=False)
    eps_d = P("eps", [t_steps, Z, BL], dt.bfloat16, isOutput=False)

    mu_o = P("mu_out", [STATE, NT], dt.float32, isOutput=True)
    s_o = P("s_out", [STATE, NT], dt.float32, isOutput=True)

    import contextlib
    with tile.TileContext(nc) as tc, contextlib.ExitStack() as octx:
        wpool = octx.enter_context(tc.tile_pool(name="weights", bufs=1))
        spool = octx.enter_context(tc.tile_pool(name="state", bufs=1))

        # ---- load weights ----
        wz = wpool.tile([Z, 4 * H], dt.bfloat16)
        wa = wpool.tile([ACT + 1, 4 * H], dt.bfloat16)
        wh = wpool.tile([128, 4 * 4 * H], dt.bfloat16)
        w1 = wpool.tile([128, 4 * 256], dt.bfloat16)
        w2 = wpool.tile([128, 2 * 128], dt.bfloat16)
        wzz = wpool.tile([128, 2 * Z], dt.bfloat16)
        b1 = wpool.tile([128, 2], dt.float32)
        b2 = wpool.tile([128, 1], dt.float32)
        bzlo = wpool.tile([128, 1], dt.float32)
        bzhi = wpool.tile([128, 1], dt.float32)
        nc.sync.dma_start(out=wz[:], in_=wz_d[:])
        nc.sync.dma_start(out=wa[:], in_=wa_d[:])
        for k in range(4):
            nc.sync.dma_start(
                out=wh[:, 2048 * k:2048 * (k + 1)], in_=wh_d[k, :, :])
            nc.sync.dma_start(
                out=w1[:, 256 * k:256 * (k + 1)], in_=w1_d[k, :, :])
        for k in range(2):
            nc.sync.dma_start(
                out=w2[:, 128 * k:128 * (k + 1)], in_=w2_d[k, :, :])
        nc.sync.dma_start(out=wzz[:], in_=wzz_d[:])
        nc.sync.dma_start(out=b1[:], in_=b1_d[:])
        nc.sync.dma_start(out=b2[:], in_=b2_d[:])
        nc.sync.dma_start(out=bzlo[:], in_=bzlo_d[:])
        nc.sync.dma_start(out=bzhi[:], in_=bzhi_d[:])

        a_aug = wpool.tile([ACT + 1, NT], dt.bfloat16)
        nc.sync.dma_start(out=a_aug[:], in_=a_d[:])

        # ---- state ----
        h2 = spool.tile([128, H], dt.bfloat16)       # 2*h
        cd = spool.tile([128, H], dt.float32)        # 2*c
        z_all = spool.tile([128, NT], dt.bfloat16)   # sampled z per step
        nc.vector.memset(h2[:], 0.0)
        nc.vector.memset(cd[:], 0.0)

        with contextlib.ExitStack() as ctx:
            gpool = ctx.enter_context(tc.tile_pool(name="gwork", bufs=2))
            epool = ctx.enter_context(tc.tile_pool(name="eps", bufs=4))
            gpsum = ctx.enter_context(
                tc.tile_pool(name="gpsum", bufs=1, space="PSUM"))
            epsum = ctx.enter_context(
                tc.tile_pool(name="epsum", bufs=2, space="PSUM"))

            # ---- recurrence ----
            for t in range(t_steps):
                eps_t = epool.tile([Z, BL], dt.bfloat16, tag="eps")
                nc.sync.dma_start(out=eps_t[:], in_=eps_d[t, :, :])

                pg = gpsum.tile([128, 4 * H], dt.float32, tag="gates")
                for m in range(16):
                    ms = slice(128 * m, 128 * (m + 1))
                    out = pg[:, ms]
                    first = True
                    if t > 0:
                        nc.tensor.matmul(
                            out, wz[:, ms],
                            z_all[:, BL * (t - 1):BL * t],
                            start=True, stop=False)
                        first = False
                    nc.tensor.matmul(
                        out, wa[:, ms], a_aug[:, BL * t:BL * (t + 1)],
                        start=first, stop=False)
                    for k in range(4):
                        nc.tensor.matmul(
                            out, wh[:, 2048 * k + 128 * m:2048 * k + 128 * (m + 1)],
                            h2[:, 128 * k:128 * (k + 1)],
                            start=False, stop=(k == 3))

                # tanh over all gates (i,f,o rows pre-scaled by 0.5 on host)
                tg = gpool.tile([128, 4 * H], dt.float32, tag="tanh_g")
                for bank in range(4):
                    bs = slice(512 * bank, 512 * (bank + 1))
                    nc.scalar.activation(tg[:, bs], pg[:, bs], AF.Tanh)

                t_i = tg[:, 0:512]
                t_f = tg[:, 512:1024]
                t_o = tg[:, 1024:1536]
                t_g = tg[:, 1536:2048]

                tmp1 = gpool.tile([128, H], dt.float32, tag="tmp1")
                tmp2 = gpool.tile([128, H], dt.float32, tag="tmp2")
                # tmp1 = (1+tanh(f/2)) * cD ; tmp2 = (1+tanh(i/2)) * g
                nc.vector.scalar_tensor_tensor(
                    tmp1[:], t_f, 1.0, cd[:], OP.add, OP.mult)
                nc.vector.scalar_tensor_tensor(
                    tmp2[:], t_i, 1.0, t_g, OP.add, OP.mult)
                # cD = 0.5*tmp1 + tmp2   (= 2*c_new)
                nc.vector.scalar_tensor_tensor(
                    cd[:], tmp1[:], 0.5, tmp2[:], OP.mult, OP.add)
                # tc = tanh(c) = tanh(0.5*cD)
                tcn = gpool.tile([128, H], dt.float32, tag="tanh_c")
                nc.scalar.activation(tcn[:], cd[:], AF.Tanh, scale=0.5)
                # h2 = (1+tanh(o/2)) * tc  (= 2*h)
                nc.vector.scalar_tensor_tensor(
                    h2[:], t_o, 1.0, tcn[:], OP.add, OP.mult)

                # encoder: e1 = relu(0.5*W1 @ h2 + b1)  [256]
                pe = epsum.tile([128, 384], dt.float32, tag="enc_a")
                for m in range(2):
                    out = pe[:, 128 * m:128 * (m + 1)]
                    for k in range(4):
                        nc.tensor.matmul(
                            out, w1[:, 256 * k + 128 * m:256 * k + 128 * (m + 1)],
                            h2[:, 128 * k:128 * (k + 1)],
                            start=(k == 0), stop=(k == 3))
                e1 = gpool.tile([128, 256], dt.bfloat16, tag="e1")
                for m in range(2):
                    nc.scalar.activation(
                        e1[:, 128 * m:128 * (m + 1)],
                        pe[:, 128 * m:128 * (m + 1)],
                        AF.Relu, bias=b1[:, m:m + 1])
                # e2 = relu(W2 @ e1 + b2)  [128]
                out = pe[:, 256:384]
                for k in range(2):
                    nc.tensor.matmul(
                        out, w2[:, 128 * k:128 * (k + 1)],
                        e1[:, 128 * k:128 * (k + 1)],
                        start=(k == 0), stop=(k == 1))
                e2 = gpool.tile([128, 128], dt.bfloat16, tag="e2")
                nc.scalar.activation(e2[:], out, AF.Relu, bias=b2[:])
                # zz = WZ @ e2 -> [z_loc | z_scale_pre]
                pz = epsum.tile([128, 2 * Z], dt.float32, tag="enc_b")
                nc.tensor.matmul(pz[:, 0:Z], wzz[:, 0:Z], e2[:],
                                 start=True, stop=True)
                nc.tensor.matmul(pz[:, Z:2 * Z], wzz[:, Z:2 * Z], e2[:],
                                 start=True, stop=True)
                # z_scale = exp(zz_hi + (1+bz_hi))
                zsc = gpool.tile([Z, BL], dt.float32, tag="zsc")
                nc.scalar.activation(zsc[:], pz[:, Z:2 * Z], AF.Exp,
                                     bias=bzhi[:])
                zse = gpool.tile([Z, BL], dt.float32, tag="zse")
                nc.vector.tensor_tensor(zse[:], zsc[:], eps_t[:], OP.mult)
                # z = (zz_lo + bz_lo) + zsc*eps
                nc.vector.scalar_tensor_tensor(
                    z_all[:, BL * t:BL * (t + 1)],
                    pz[:, 0:Z], bzlo[:], zse[:], OP.add, OP.add)

        # ---- decoder, batched over all steps ----
        with contextlib.ExitStack() as ctx:
            # reuse z_all/spool & weight pool tiles? pools above released;
            # keep decoder self-contained: reload small dec weights
            dwp = ctx.enter_context(tc.tile_pool(name="dec_w", bufs=1))
            dsp = ctx.enter_context(tc.tile_pool(name="dec_s", bufs=3))
            dps = ctx.enter_context(
                tc.tile_pool(name="dec_p", bufs=1, space="PSUM"))

            dw1z = dwp.tile([Z, 64], dt.bfloat16)
            dw1o = dwp.tile([STATE + 1, 64], dt.bfloat16)
            dw2 = dwp.tile([64, 64], dt.bfloat16)
            dw3 = dwp.tile([64, 32], dt.bfloat16)
            dw4 = dwp.tile([32, 16], dt.bfloat16)
            mw = dwp.tile([16, STATE], dt.bfloat16)
            sw = dwp.tile([16, STATE], dt.bfloat16)
            db2 = dwp.tile([64, 1], dt.float32)
            db3 = dwp.tile([32, 1], dt.float32)
            db4 = dwp.tile([16, 1], dt.float32)
            mb = dwp.tile([STATE, 1], dt.float32)
            sb = dwp.tile([STATE, 1], dt.float32)
            obs = dwp.tile([STATE + 1, 512], dt.bfloat16)
            for tdst, tsrc in [(dw1z, dw1z_d), (dw1o, dw1o_d), (dw2, dw2_d),
                               (dw3, dw3_d), (dw4, dw4_d), (mw, mw_d),
                               (sw, sw_d), (db2, db2_d), (db3, db3_d),
                               (db4, db4_d), (mb, mb_d), (sb, sb_d),
                               (obs, obs_d)]:
                nc.sync.dma_start(out=tdst[:], in_=tsrc[:])

            nchunk = NT // 512
            for cidx in range(nchunk):
                cs = slice(512 * cidx, 512 * (cidx + 1))
                zc = z_all[:, cs]
                p1 = dps.tile([64, 512], dt.float32, tag="d1p")
                nc.tensor.matmul(p1[:], dw1z[:], zc, start=True, stop=False)
                nc.tensor.matmul(p1[:], dw1o[:], obs[:], start=False, stop=True)
                d1 = dsp.tile([64, 512], dt.bfloat16, tag="d1")
                nc.scalar.activation(d1[:], p1[:], AF.Relu)
                p2 = dps.tile([64, 512], dt.float32, tag="d2p")
                nc.tensor.matmul(p2[:], dw2[:], d1[:], start=True, stop=True)
                d2 = dsp.tile([64, 512], dt.bfloat16, tag="d2")
                nc.scalar.activation(d2[:], p2[:], AF.Relu, bias=db2[:])
                p3 = dps.tile([32, 512], dt.float32, tag="d3p")
                nc.tensor.matmul(p3[:], dw3[:], d2[:], start=True, stop=True)
                d3 = dsp.tile([32, 512], dt.bfloat16, tag="d3")
                nc.scalar.activation(d3[:], p3[:], AF.Relu, bias=db3[:])
                p4 = dps.tile([16, 512], dt.float32, tag="d4p")
                nc.tensor.matmul(p4[:], dw4[:], d3[:], start=True, stop=True)
                d4 = dsp.tile([16, 512], dt.bfloat16, tag="d4")
                nc.scalar.activation(d4[:], p4[:], AF.Relu, bias=db4[:])
                pm = dps.tile([STATE, 512], dt.float32, tag="mup")
                nc.tensor.matmul(pm[:], mw[:], d4[:], start=True, stop=True)
                ps = dps.tile([STATE, 512], dt.float32, tag="sgp")
                nc.tensor.matmul(ps[:], sw[:], d4[:], start=True, stop=True)
                mu_sb = dsp.tile([STATE, 512], dt.float32, tag="mu")
                s_sb = dsp.tile([STATE, 512], dt.float32, tag="sg")
                nc.vector.tensor_scalar_add(mu_sb[:], pm[:], mb[:])
                nc.vector.tensor_scalar_add(s_sb[:], ps[:], sb[:])
                nc.sync.dma_start(out=mu_o[:, cs], in_=mu_sb[:])
                nc.sync.dma_start(out=s_o[:, cs], in_=s_sb[:])

    _split_multi_waits(nc)
    return nc


def _prep_host(inputs, t_steps):
    """Host-side weight/data prep -> per-core in_maps."""
    f32 = np.float32
    x, a = inputs["x"], inputs["a"]
    W_ih, W_hh = f32(inputs["W_ih"]), f32(inputs["W_hh"])
    b_g = f32(inputs["b_ih"]) + f32(inputs["b_hh"])

    # reorder gates (i,f,g,o) -> (i,f,o,g); scale i,f,o rows (and bias) by 0.5
    perm = np.concatenate([np.arange(0, H), np.arange(H, 2 * H),
                           np.arange(3 * H, 4 * H), np.arange(2 * H, 3 * H)])
    sc = np.ones(4 * H, f32)
    sc[:3 * H] = 0.5
    W_ih_r = W_ih[perm] * sc[:, None]
    W_hh_r = W_hh[perm] * sc[:, None]
    b_r = b_g[perm] * sc

    wz = np.ascontiguousarray(W_ih_r[:, ACT:].T).astype(bf16)        # [128, 2048]
    wa = np.ascontiguousarray(
        np.concatenate([W_ih_r[:, :ACT], b_r[:, None]], axis=1).T).astype(bf16)
    wh = np.ascontiguousarray(
        (0.5 * W_hh_r).T.reshape(4, 128, 4 * H)).astype(bf16)
    w1 = np.ascontiguousarray(
        (0.5 * f32(inputs["enc_w1"])).T.reshape(4, 128, 256)).astype(bf16)
    b1 = np.ascontiguousarray(f32(inputs["enc_b1"]).reshape(2, 128).T)
    w2 = np.ascontiguousarray(
        f32(inputs["enc_w2"]).T.reshape(2, 128, 128)).astype(bf16)
    b2 = f32(inputs["enc_b2"]).reshape(128, 1)
    wzz = np.ascontiguousarray(f32(inputs["enc_wz"]).T).astype(bf16)  # [128, 256]
    bzlo = f32(inputs["enc_bz"])[:Z].reshape(Z, 1)
    bzhi = (1.0 + f32(inputs["enc_bz"])[Z:]).reshape(Z, 1)

    dec_w1 = f32(inputs["dec_w1"])
    dw1z = np.ascontiguousarray(dec_w1[:, STATE:].T).astype(bf16)    # [128, 64]
    dw1o = np.ascontiguousarray(
        np.concatenate([dec_w1[:, :STATE], f32(inputs["dec_b1"])[:, None]],
                       axis=1).T).astype(bf16)                       # [22, 64]
    dw2 = np.ascontiguousarray(f32(inputs["dec_w2"]).T).astype(bf16)
    db2 = f32(inputs["dec_b2"]).reshape(64, 1)
    dw3 = np.ascontiguousarray(f32(inputs["dec_w3"]).T).astype(bf16)
    db3 = f32(inputs["dec_b3"]).reshape(32, 1)
    dw4 = np.ascontiguousarray(f32(inputs["dec_w4"]).T).astype(bf16)
    db4 = f32(inputs["dec_b4"]).reshape(16, 1)
    mw = np.ascontiguousarray(f32(inputs["mu_w"]).T).astype(bf16)
    mb = f32(inputs["mu_b"]).reshape(STATE, 1)
    sw = np.ascontiguousarray(f32(inputs["sig_w"]).T).astype(bf16)
    sb = (1.0 + f32(inputs["sig_b"])).reshape(STATE, 1)

    # eps via jax CPU (exact reference PRNG)
    import jax
    with jax.default_device(jax.devices("cpu")[0]):
        eps = np.asarray(jax.random.normal(
            jax.random.key(42), (T, B, Z), dtype=jax.numpy.float32))

    shared = dict(wz=wz, wa=wa, wh=wh, w1=w1, b1=b1, w2=w2, b2=b2, wzz=wzz,
                  bzlo=bzlo, bzhi=bzhi, dw1z=dw1z, dw1o=dw1o, dw2=dw2,
                  db2=db2, dw3=dw3, db3=db3, dw4=dw4, db4=db4, mw=mw, mb=mb,
                  sw=sw, sb=sb)

    in_maps = []
    ones = np.ones((1, t_steps * BL), f32)
    for ci in range(NCORES):
        bs = slice(ci * BL, (ci + 1) * BL)
        # a_aug: [9, T*BL], free index = t*BL + b
        a_c = np.ascontiguousarray(
            f32(a[bs, :t_steps, :]).transpose(2, 1, 0).reshape(
                ACT, t_steps * BL))
        # careful: transpose(2,1,0) gives [ACT, T, BL] -> flatten t-major ok
        a_aug = np.concatenate([a_c, ones], axis=0).astype(bf16)
        eps_c = np.ascontiguousarray(
            eps[:t_steps, bs, :].transpose(0, 2, 1)).astype(bf16)  # [T, Z, BL]
        obs_c = f32(x[bs, 0, :]).T                                 # [21, BL]
        obs_rep = np.concatenate(
            [np.tile(obs_c, (1, 512 // BL)),
             np.ones((1, 512), f32)], axis=0).astype(bf16)
        m = dict(shared)
        m.update(a_aug=a_aug, eps=eps_c, obs_rep=obs_rep)
        in_maps.append(m)
    return in_maps


def _run(inputs, t_steps=T):
    from concourse.bass_utils import run_bass_kernel_spmd

    key = ("nc", t_steps)
    if key not in _CACHE:
        _CACHE[key] = _build_nc(t_steps)
    nc = _CACHE[key]
    in_maps = _prep_host(inputs, t_steps)
    res = run_bass_kernel_spmd(nc, in_maps, list(range(NCORES)),
                               trace=False)
    return res.results


def kernel(**inputs):
    t_steps = T
    results = _run(inputs, t_steps)

    y = np.float32(inputs["y"])
    su2 = 0.0
    ss = 0.0
    sabs = 0.0
    ssd = 0.0
    n_el = NCORES * STATE * t_steps * BL
    for ci in range(NCORES):
        bs = slice(ci * BL, (ci + 1) * BL)
        mu = results[ci]["mu_out"].astype(np.float64)     # [21, T*BL]
        s = results[ci]["s_out"].astype(np.float64)       # log(sigma)
        y_c = y[bs, :t_steps, :].transpose(2, 1, 0).reshape(
            STATE, t_steps * BL).astype(np.float64)
        sd = np.exp(s)
        u = (y_c - mu) / sd
        su2 += (u * u).sum()
        ss += s.sum()
        sabs += np.abs(mu - y_c).sum()
        ssd += sd.sum()
    n_tb = NCORES * t_steps * BL
    out1 = (0.5 * su2 + ss) / n_tb + STATE * LOG_SQRT_2PI
    out2 = sabs / n_el
    out3 = ssd / n_el
    return (np.float32(out1), np.float32(out2), np.float32(out3))


if __name__ == "__main__":
    import jax
    with jax.default_device(jax.devices("cpu")[0]):
        import reference as R
        inputs = {k: np.asarray(v) for k, v in R.setup_inputs().items()}
    out = kernel(**inputs)
    print("kernel:", [float(o) for o in out])


# revision 6
# speedup vs baseline: 1.1711x; 1.1711x over previous
"""Trainium2 Bass kernel for nn_MDNSeqModel: LSTM + encoder recurrence with
MDN decoder, data-parallel over batch across 8 NeuronCores.

Layout: feature-major activations [features(partitions), batch(free)],
batch 1024 sharded 8 ways -> 128 batch columns per core (= free dim of every
recurrent matmul). All matmul operands bf16, PSUM accumulation fp32,
elementwise fp32. Sigmoid computed as 0.5 + 0.5*tanh(x/2) (0.5 pre-folded
into the i/f/o weight rows) so tanh/exp/relu share one ACT table set.
State carried as cD = 2c and h2 = 2h so the 0.5 factors fold into
scalar_tensor_tensor ops and the weights that consume h.
"""
import os

import numpy as np
import ml_dtypes

STATE, ACT, Z, H = 21, 8, 128, 512
B, T = 1024, 128
NCORES = 8
BL = B // NCORES          # batch per core (free dim)
LOG_SQRT_2PI = 0.9189385332046727

bf16 = ml_dtypes.bfloat16

_CACHE = {}


def _split_multi_waits(nc, max_waits=1):
    """This walrus build rejects instructions carrying more than one sync-wait
    command; Tile's semaphore pass emits up to ~4 per instruction. Hoist the
    extras onto single-wait NOPs inserted just before, on the same engine
    (each engine executes its own stream in program order, so the semantics
    are identical)."""
    import concourse.mybir as mybir

    n_nops = 0
    for f in nc.m.functions:
        for bb in f.blocks:
            insts = bb.instructions
            out = []
            changed = False
            for ins in insts:
                si = ins.sync_info
                waits = list(si.on_wait) if si is not None else []
                if len(waits) > max_waits:
                    changed = True
                    extra = waits[:-max_waits]
                    for k, w in enumerate(extra):
                        nop = mybir.InstNoOp(
                            name=f"{ins.name}-wsplit{k}", engine=ins.engine)
                        nop.sync_info = mybir.SyncInfo(
                            on_update=[], on_wait=[w])
                        out.append(nop)
                        n_nops += 1
                    while len(si.on_wait) > max_waits:
                        si.on_wait.pop(0)
                out.append(ins)
            if changed:
                bb.instructions = out
    return n_nops


def _build_nc(t_steps):
    """Build the Bass module (same NEFF for all cores; SPMD over in_maps)."""
    import concourse.bass as bass
    import concourse.mybir as mybir
    import concourse.tile as tile

    dt = mybir.dt
    AF = mybir.ActivationFunctionType
    OP = mybir.AluOpType
    NT = t_steps * BL       # decoder free length

    nc = bass.Bass()
    P = nc.declare_dram_parameter

    # ---- inputs (per-core, host-prepped) ----
    wz_d = P("wz", [Z, 4 * H], dt.bfloat16, isOutput=False)        # W_ih z-part ^T
    wa_d = P("wa", [ACT + 1, 4 * H], dt.bfloat16, isOutput=False)  # [W_ih a-part | b]^T
    wh_d = P("wh", [4, 128, 4 * H], dt.bfloat16, isOutput=False)   # (0.5*W_hh)^T k-chunks
    w1_d = P("w1", [4, 128, 256], dt.bfloat16, isOutput=False)     # (0.5*enc_w1)^T
    b1_d = P("b1", [128, 2], dt.float32, isOutput=False)
    w2_d = P("w2", [2, 128, 128], dt.bfloat16, isOutput=False)     # enc_w2^T
    b2_d = P("b2", [128, 1], dt.float32, isOutput=False)
    wzz_d = P("wzz", [128, 2 * Z], dt.bfloat16, isOutput=False)    # enc_wz^T
    bzlo_d = P("bzlo", [128, 1], dt.float32, isOutput=False)
    bzhi_d = P("bzhi", [128, 1], dt.float32, isOutput=False)       # 1 + bz_hi
    dw1z_d = P("dw1z", [Z, 64], dt.bfloat16, isOutput=False)
    dw1o_d = P("dw1o", [STATE + 1, 64], dt.bfloat16, isOutput=False)  # [w1_obs | b1]^T
    dw2_d = P("dw2", [64, 64], dt.bfloat16, isOutput=False)
    db2_d = P("db2", [64, 1], dt.float32, isOutput=False)
    dw3_d = P("dw3", [64, 32], dt.bfloat16, isOutput=False)
    db3_d = P("db3", [32, 1], dt.float32, isOutput=False)
    dw4_d = P("dw4", [32, 16], dt.bfloat16, isOutput=False)
    db4_d = P("db4", [16, 1], dt.float32, isOutput=False)
    mw_d = P("mw", [16, STATE], dt.bfloat16, isOutput=False)
    mb_d = P("mb", [STATE, 1], dt.float32, isOutput=False)
    sw_d = P("sw", [16, STATE], dt.bfloat16, isOutput=False)
    sb_d = P("sb", [STATE, 1], dt.float32, isOutput=False)         # 1 + sig_b
    a_d = P("a_aug", [ACT + 1, NT], dt.bfloat16, isOutput=False)   # [a_t^T; 1]
    obs_d = P("obs_rep", [STATE + 1, 512], dt.bfloat16, isOutput=False)
    eps_d = P("eps", [t_steps, Z, BL], dt.bfloat16, isOutput=False)

    mu_o = P("mu_out", [STATE, NT], dt.float32, isOutput=True)
    s_o = P("s_out", [STATE, NT], dt.float32, isOutput=True)

    import contextlib
    with tile.TileContext(nc) as tc, contextlib.ExitStack() as octx:
        wpool = octx.enter_context(tc.tile_pool(name="weights", bufs=1))
        spool = octx.enter_context(tc.tile_pool(name="state", bufs=1))

        # ---- load weights ----
        wz = wpool.tile([Z, 4 * H], dt.bfloat16)
        wa = wpool.tile([ACT + 1, 4 * H], dt.bfloat16)
        wh = wpool.tile([128, 4 * 4 * H], dt.bfloat16)
        w1 = wpool.tile([128, 4 * 256], dt.bfloat16)
        w2 = wpool.tile([128, 2 * 128], dt.bfloat16)
        wzz = wpool.tile([128, 2 * Z], dt.bfloat16)
        b1 = wpool.tile([128, 2], dt.float32)
        b2 = wpool.tile([128, 1], dt.float32)
        bzlo = wpool.tile([128, 1], dt.float32)
        bzhi = wpool.tile([128, 1], dt.float32)
        nc.sync.dma_start(out=wz[:], in_=wz_d[:])
        nc.sync.dma_start(out=wa[:], in_=wa_d[:])
        for k in range(4):
            nc.sync.dma_start(
                out=wh[:, 2048 * k:2048 * (k + 1)], in_=wh_d[k, :, :])
            nc.sync.dma_start(
                out=w1[:, 256 * k:256 * (k + 1)], in_=w1_d[k, :, :])
        for k in range(2):
            nc.sync.dma_start(
                out=w2[:, 128 * k:128 * (k + 1)], in_=w2_d[k, :, :])
        nc.sync.dma_start(out=wzz[:], in_=wzz_d[:])
        nc.sync.dma_start(out=b1[:], in_=b1_d[:])
        nc.sync.dma_start(out=b2[:], in_=b2_d[:])
        nc.sync.dma_start(out=bzlo[:], in_=bzlo_d[:])
        nc.sync.dma_start(out=bzhi[:], in_=bzhi_d[:])

        a_aug = wpool.tile([ACT + 1, NT], dt.bfloat16)
        nc.sync.dma_start(out=a_aug[:], in_=a_d[:])

        # ---- state ----
        # batch-major LSTM state (batch on partitions, features free)
        h2b = spool.tile([128, H], dt.bfloat16)      # 2*h, batch-major
        h2f = spool.tile([128, H], dt.bfloat16)      # 2*h, feature-major
        cd = spool.tile([128, H], dt.float32)        # 2*c, batch-major
        z_all = spool.tile([128, NT], dt.bfloat16)   # sampled z (feature-major)
        ident = spool.tile([128, 128], dt.bfloat16)
        nc.vector.memset(h2f[:], 0.0)
        nc.vector.memset(cd[:], 0.0)
        from concourse.masks import make_identity
        make_identity(nc, ident[:])

        with contextlib.ExitStack() as ctx:
            gpool = ctx.enter_context(tc.tile_pool(name="gwork", bufs=2))
            epool = ctx.enter_context(tc.tile_pool(name="eps", bufs=4))
            gpsum = ctx.enter_context(
                tc.tile_pool(name="gpsum", bufs=1, space="PSUM"))
            epsum = ctx.enter_context(
                tc.tile_pool(name="epsum", bufs=1, space="PSUM"))
            tpsum = ctx.enter_context(
                tc.tile_pool(name="tpsum", bufs=2, space="PSUM"))

            # ---- recurrence ----
            for t in range(t_steps):
                eps_t = epool.tile([Z, BL], dt.bfloat16, tag="eps")
                nc.sync.dma_start(out=eps_t[:], in_=eps_d[t, :, :])

                # gates batch-major: psum[b, g] — stationary activations
                # (feature-major), weights moving in 512-wide windows.
                pg = gpsum.tile([128, 4 * H], dt.float32, tag="gates")
                pairs = []
                if t > 0:
                    pairs.append((z_all[:, BL * (t - 1):BL * t], wz, 0))
                pairs.append((a_aug[:, BL * t:BL * (t + 1)], wa, 0))
                for k in range(4):
                    pairs.append((h2f[:, 128 * k:128 * (k + 1)], wh, 2048 * k))
                for ki, (stat, wmov, off) in enumerate(pairs):
                    for n in range(4):
                        nc.tensor.matmul(
                            pg[:, 512 * n:512 * (n + 1)], stat,
                            wmov[:, off + 512 * n:off + 512 * (n + 1)],
                            start=(ki == 0), stop=(ki == len(pairs) - 1))

                # tanh over all gates (i,f,o cols pre-scaled by 0.5 on host)
                tg = gpool.tile([128, 4 * H], dt.float32, tag="tanh_g")
                nc.scalar.activation(tg[:, 0:1024], pg[:, 0:1024], AF.Tanh)
                nc.scalar.activation(tg[:, 1024:2048], pg[:, 1024:2048],
                                     AF.Tanh)

                t_i = tg[:, 0:512]
                t_f = tg[:, 512:1024]
                t_o = tg[:, 1024:1536]
                t_g = tg[:, 1536:2048]

                tmp1 = gpool.tile([128, H], dt.float32, tag="tmp1")
                tmp2 = gpool.tile([128, H], dt.float32, tag="tmp2")
                # tmp1 = (1+tanh(f/2)) * cD ; tmp2 = (1+tanh(i/2)) * g
                nc.vector.scalar_tensor_tensor(
                    tmp1[:], t_f, 1.0, cd[:], OP.add, OP.mult)
                nc.vector.scalar_tensor_tensor(
                    tmp2[:], t_i, 1.0, t_g, OP.add, OP.mult)
                # cD = 0.5*tmp1 + tmp2   (= 2*c_new)
                nc.vector.scalar_tensor_tensor(
                    cd[:], tmp1[:], 0.5, tmp2[:], OP.mult, OP.add)
                # tc = tanh(c) = tanh(0.5*cD)
                tcn = gpool.tile([128, H], dt.float32, tag="tanh_c")
                nc.scalar.activation(tcn[:], cd[:], AF.Tanh, scale=0.5)
                # h2 = (1+tanh(o/2)) * tc  (= 2*h)
                nc.vector.scalar_tensor_tensor(
                    h2b[:], t_o, 1.0, tcn[:], OP.add, OP.mult)

                # transpose h2 batch-major -> feature-major via PE
                for k in range(4):
                    ptr = tpsum.tile([128, 128], dt.bfloat16, tag="tr")
                    nc.tensor.transpose(
                        ptr[:], h2b[:, 128 * k:128 * (k + 1)], ident[:])
                    nc.scalar.copy(h2f[:, 128 * k:128 * (k + 1)], ptr[:])

                # encoder (feature-major): e1 = relu(0.5*W1 @ h2 + b1)  [256]
                pe = epsum.tile([128, 384], dt.float32, tag="enc_a")
                for m in range(2):
                    out = pe[:, 128 * m:128 * (m + 1)]
                    for k in range(4):
                        nc.tensor.matmul(
                            out, w1[:, 256 * k + 128 * m:256 * k + 128 * (m + 1)],
                            h2f[:, 128 * k:128 * (k + 1)],
                            start=(k == 0), stop=(k == 3))
                e1 = gpool.tile([128, 256], dt.bfloat16, tag="e1")
                for m in range(2):
                    nc.vector.tensor_scalar(
                        e1[:, 128 * m:128 * (m + 1)],
                        pe[:, 128 * m:128 * (m + 1)],
                        b1[:, m:m + 1], 0.0, OP.add, OP.max)
                # e2 = relu(W2 @ e1 + b2)  [128]
                out = pe[:, 256:384]
                for k in range(2):
                    nc.tensor.matmul(
                        out, w2[:, 128 * k:128 * (k + 1)],
                        e1[:, 128 * k:128 * (k + 1)],
                        start=(k == 0), stop=(k == 1))
                e2 = gpool.tile([128, 128], dt.bfloat16, tag="e2")
                nc.vector.tensor_scalar(
                    e2[:], out, b2[:], 0.0, OP.add, OP.max)
                # zz = WZ @ e2 -> [z_loc | z_scale_pre]
                pz = epsum.tile([128, 2 * Z], dt.float32, tag="enc_b")
                nc.tensor.matmul(pz[:, 0:Z], wzz[:, 0:Z], e2[:],
                                 start=True, stop=True)
                nc.tensor.matmul(pz[:, Z:2 * Z], wzz[:, Z:2 * Z], e2[:],
                                 start=True, stop=True)
                # z_scale = exp(zz_hi + (1+bz_hi))
                zsc = gpool.tile([Z, BL], dt.float32, tag="zsc")
                nc.scalar.activation(zsc[:], pz[:, Z:2 * Z], AF.Exp,
                                     bias=bzhi[:])
                zse = gpool.tile([Z, BL], dt.float32, tag="zse")
                nc.vector.tensor_tensor(zse[:], zsc[:], eps_t[:], OP.mult)
                # z = (zz_lo + bz_lo) + zsc*eps
                nc.vector.scalar_tensor_tensor(
                    z_all[:, BL * t:BL * (t + 1)],
                    pz[:, 0:Z], bzlo[:], zse[:], OP.add, OP.add)

        # ---- decoder, batched over all steps ----
        with contextlib.ExitStack() as ctx:
            # reuse z_all/spool & weight pool tiles? pools above released;
            # keep decoder self-contained: reload small dec weights
            dwp = ctx.enter_context(tc.tile_pool(name="dec_w", bufs=1))
            dsp = ctx.enter_context(tc.tile_pool(name="dec_s", bufs=3))
            dps = ctx.enter_context(
                tc.tile_pool(name="dec_p", bufs=1, space="PSUM"))

            dw1z = dwp.tile([Z, 64], dt.bfloat16)
            dw1o = dwp.tile([STATE + 1, 64], dt.bfloat16)
            dw2 = dwp.tile([64, 64], dt.bfloat16)
            dw3 = dwp.tile([64, 32], dt.bfloat16)
            dw4 = dwp.tile([32, 16], dt.bfloat16)
            mw = dwp.tile([16, STATE], dt.bfloat16)
            sw = dwp.tile([16, STATE], dt.bfloat16)
            db2 = dwp.tile([64, 1], dt.float32)
            db3 = dwp.tile([32, 1], dt.float32)
            db4 = dwp.tile([16, 1], dt.float32)
            mb = dwp.tile([STATE, 1], dt.float32)
            sb = dwp.tile([STATE, 1], dt.float32)
            obs = dwp.tile([STATE + 1, 512], dt.bfloat16)
            for tdst, tsrc in [(dw1z, dw1z_d), (dw1o, dw1o_d), (dw2, dw2_d),
                               (dw3, dw3_d), (dw4, dw4_d), (mw, mw_d),
                               (sw, sw_d), (db2, db2_d), (db3, db3_d),
                               (db4, db4_d), (mb, mb_d), (sb, sb_d),
                               (obs, obs_d)]:
                nc.sync.dma_start(out=tdst[:], in_=tsrc[:])

            nchunk = NT // 512
            for cidx in range(nchunk):
                cs = slice(512 * cidx, 512 * (cidx + 1))
                zc = z_all[:, cs]
                p1 = dps.tile([64, 512], dt.float32, tag="d1p")
                nc.tensor.matmul(p1[:], dw1z[:], zc, start=True, stop=False)
                nc.tensor.matmul(p1[:], dw1o[:], obs[:], start=False, stop=True)
                d1 = dsp.tile([64, 512], dt.bfloat16, tag="d1")
                nc.scalar.activation(d1[:], p1[:], AF.Relu)
                p2 = dps.tile([64, 512], dt.float32, tag="d2p")
                nc.tensor.matmul(p2[:], dw2[:], d1[:], start=True, stop=True)
                d2 = dsp.tile([64, 512], dt.bfloat16, tag="d2")
                nc.scalar.activation(d2[:], p2[:], AF.Relu, bias=db2[:])
                p3 = dps.tile([32, 512], dt.float32, tag="d3p")
                nc.tensor.matmul(p3[:], dw3[:], d2[:], start=True, stop=True)
                d3 = dsp.tile([32, 512], dt.bfloat16, tag="d3")
                nc.scalar.activation(d3[:], p3[:], AF.Relu, bias=db3[:])
                p4 = dps.tile([16, 512], dt.float32, tag="d4p")
                nc.tensor.matmul(p4[:], dw4[:], d3[:], start=True, stop=True)
                d4 = dsp.tile([16, 512], dt.bfloat16, tag="d4")
                nc.scalar.activation(d4[:], p4[:], AF.Relu, bias=db4[:])
                pm = dps.tile([STATE, 512], dt.float32, tag="mup")
                nc.tensor.matmul(pm[:], mw[:], d4[:], start=True, stop=True)
                ps = dps.tile([STATE, 512], dt.float32, tag="sgp")
                nc.tensor.matmul(ps[:], sw[:], d4[:], start=True, stop=True)
                mu_sb = dsp.tile([STATE, 512], dt.float32, tag="mu")
                s_sb = dsp.tile([STATE, 512], dt.float32, tag="sg")
                nc.vector.tensor_scalar_add(mu_sb[:], pm[:], mb[:])
                nc.vector.tensor_scalar_add(s_sb[:], ps[:], sb[:])
                nc.sync.dma_start(out=mu_o[:, cs], in_=mu_sb[:])
                nc.sync.dma_start(out=s_o[:, cs], in_=s_sb[:])

    _split_multi_waits(nc)
    return nc


def _prep_host(inputs, t_steps):
    """Host-side weight/data prep -> per-core in_maps."""
    f32 = np.float32
    x, a = inputs["x"], inputs["a"]
    W_ih, W_hh = f32(inputs["W_ih"]), f32(inputs["W_hh"])
    b_g = f32(inputs["b_ih"]) + f32(inputs["b_hh"])

    # reorder gates (i,f,g,o) -> (i,f,o,g); scale i,f,o rows (and bias) by 0.5
    perm = np.concatenate([np.arange(0, H), np.arange(H, 2 * H),
                           np.arange(3 * H, 4 * H), np.arange(2 * H, 3 * H)])
    sc = np.ones(4 * H, f32)
    sc[:3 * H] = 0.5
    W_ih_r = W_ih[perm] * sc[:, None]
    W_hh_r = W_hh[perm] * sc[:, None]
    b_r = b_g[perm] * sc

    wz = np.ascontiguousarray(W_ih_r[:, ACT:].T).astype(bf16)        # [128, 2048]
    wa = np.ascontiguousarray(
        np.concatenate([W_ih_r[:, :ACT], b_r[:, None]], axis=1).T).astype(bf16)
    wh = np.ascontiguousarray(
        (0.5 * W_hh_r).T.reshape(4, 128, 4 * H)).astype(bf16)
    w1 = np.ascontiguousarray(
        (0.5 * f32(inputs["enc_w1"])).T.reshape(4, 128, 256)).astype(bf16)
    b1 = np.ascontiguousarray(f32(inputs["enc_b1"]).reshape(2, 128).T)
    w2 = np.ascontiguousarray(
        f32(inputs["enc_w2"]).T.reshape(2, 128, 128)).astype(bf16)
    b2 = f32(inputs["enc_b2"]).reshape(128, 1)
    wzz = np.ascontiguousarray(f32(inputs["enc_wz"]).T).astype(bf16)  # [128, 256]
    bzlo = f32(inputs["enc_bz"])[:Z].reshape(Z, 1)
    bzhi = (1.0 + f32(inputs["enc_bz"])[Z:]).reshape(Z, 1)

    dec_w1 = f32(inputs["dec_w1"])
    dw1z = np.ascontiguousarray(dec_w1[:, STATE:].T).astype(bf16)    # [128, 64]
    dw1o = np.ascontiguousarray(
        np.concatenate([dec_w1[:, :STATE], f32(inputs["dec_b1"])[:, None]],
                       axis=1).T).astype(bf16)                       # [22, 64]
    dw2 = np.ascontiguousarray(f32(inputs["dec_w2"]).T).astype(bf16)
    db2 = f32(inputs["dec_b2"]).reshape(64, 1)
    dw3 = np.ascontiguousarray(f32(inputs["dec_w3"]).T).astype(bf16)
    db3 = f32(inputs["dec_b3"]).reshape(32, 1)
    dw4 = np.ascontiguousarray(f32(inputs["dec_w4"]).T).astype(bf16)
    db4 = f32(inputs["dec_b4"]).reshape(16, 1)
    mw = np.ascontiguousarray(f32(inputs["mu_w"]).T).astype(bf16)
    mb = f32(inputs["mu_b"]).reshape(STATE, 1)
    sw = np.ascontiguousarray(f32(inputs["sig_w"]).T).astype(bf16)
    sb = (1.0 + f32(inputs["sig_b"])).reshape(STATE, 1)

    # eps via jax CPU (exact reference PRNG)
    import jax
    with jax.default_device(jax.devices("cpu")[0]):
        eps = np.asarray(jax.random.normal(
            jax.random.key(42), (T, B, Z), dtype=jax.numpy.float32))

    shared = dict(wz=wz, wa=wa, wh=wh, w1=w1, b1=b1, w2=w2, b2=b2, wzz=wzz,
                  bzlo=bzlo, bzhi=bzhi, dw1z=dw1z, dw1o=dw1o, dw2=dw2,
                  db2=db2, dw3=dw3, db3=db3, dw4=dw4, db4=db4, mw=mw, mb=mb,
                  sw=sw, sb=sb)

    in_maps = []
    ones = np.ones((1, t_steps * BL), f32)
    for ci in range(NCORES):
        bs = slice(ci * BL, (ci + 1) * BL)
        # a_aug: [9, T*BL], free index = t*BL + b
        a_c = np.ascontiguousarray(
            f32(a[bs, :t_steps, :]).transpose(2, 1, 0).reshape(
                ACT, t_steps * BL))
        # careful: transpose(2,1,0) gives [ACT, T, BL] -> flatten t-major ok
        a_aug = np.concatenate([a_c, ones], axis=0).astype(bf16)
        eps_c = np.ascontiguousarray(
            eps[:t_steps, bs, :].transpose(0, 2, 1)).astype(bf16)  # [T, Z, BL]
        obs_c = f32(x[bs, 0, :]).T                                 # [21, BL]
        obs_rep = np.concatenate(
            [np.tile(obs_c, (1, 512 // BL)),
             np.ones((1, 512), f32)], axis=0).astype(bf16)
        m = dict(shared)
        m.update(a_aug=a_aug, eps=eps_c, obs_rep=obs_rep)
        in_maps.append(m)
    return in_maps


def _run(inputs, t_steps=T):
    from concourse.bass_utils import run_bass_kernel_spmd

    key = ("nc", t_steps)
    if key not in _CACHE:
        _CACHE[key] = _build_nc(t_steps)
    nc = _CACHE[key]
    in_maps = _prep_host(inputs, t_steps)
    res = run_bass_kernel_spmd(nc, in_maps, list(range(NCORES)),
                               trace=False)
    return res.results


def kernel(**inputs):
    t_steps = T
    results = _run(inputs, t_steps)

    y = np.float32(inputs["y"])
    su2 = 0.0
    ss = 0.0
    sabs = 0.0
    ssd = 0.0
    n_el = NCORES * STATE * t_steps * BL
    for ci in range(NCORES):
        bs = slice(ci * BL, (ci + 1) * BL)
        mu = results[ci]["mu_out"].astype(np.float64)     # [21, T*BL]
        s = results[ci]["s_out"].astype(np.float64)       # log(sigma)
        y_c = y[bs, :t_steps, :].transpose(2, 1, 0).reshape(
            STATE, t_steps * BL).astype(np.float64)
        sd = np.exp(s)
        u = (y_c - mu) / sd
        su2 += (u * u).sum()
        ss += s.sum()
        sabs += np.abs(mu - y_c).sum()
        ssd += sd.sum()
    n_tb = NCORES * t_steps * BL
    out1 = (0.5 * su2 + ss) / n_tb + STATE * LOG_SQRT_2PI
    out2 = sabs / n_el
    out3 = ssd / n_el
    return (np.float32(out1), np.float32(out2), np.float32(out3))


if __name__ == "__main__":
    import jax
    with jax.default_device(jax.devices("cpu")[0]):
        import reference as R
        inputs = {k: np.asarray(v) for k, v in R.setup_inputs().items()}
    out = kernel(**inputs)
    print("kernel:", [float(o) for o in out])


# revision 8
# speedup vs baseline: 1.8255x; 1.5588x over previous
"""Trainium2 Bass kernel for nn_MDNSeqModel: LSTM + encoder recurrence with
MDN decoder, data-parallel over batch across 8 NeuronCores.

Batch 1024 is sharded 8 ways -> 128 batch columns per core. The LSTM gate
matmul runs batch-major (small feature-major activations stationary on the
PE, weights moving in 512-wide windows) so the per-matmul LDWEIGHTS is
hidden. Per-PSUM-window accumulation groups are ordered a-part (no deps)
-> h-part -> z-part (latest dep) so next-step matmuls overlap the previous
step's nonlinearity tail and the PE never idles long enough to re-throttle.
All matmul operands bf16, PSUM accumulation fp32, elementwise fp32.
Sigmoid computed as 0.5 + 0.5*tanh(x/2) (0.5 pre-folded into the i/f/o
weight rows) so tanh/exp share one ACT table set; relus run on the DVE.
State carried as cD = 2c and h2 = 2h so the 0.5 factors fold into
scalar_tensor_tensor ops and the weights that consume h. h2 is transposed
back to feature-major by DMA-transpose. The decoder is interleaved into the
recurrence (one 512-column chunk every 4 steps) to fill PE gaps; the final
log-prob / mean reductions run on the host in float64 from the returned
mu and log-sigma.
"""
import os

import numpy as np
import ml_dtypes

STATE, ACT, Z, H = 21, 8, 128, 512
B, T = 1024, 128
NCORES = 8
BL = B // NCORES          # batch per core (free dim)
LOG_SQRT_2PI = 0.9189385332046727

bf16 = ml_dtypes.bfloat16

_CACHE = {}


def _split_multi_waits(nc, max_waits=1):
    """This walrus build rejects instructions carrying more than one sync-wait
    command; Tile's semaphore pass emits up to ~4 per instruction. Hoist the
    extras onto single-wait NOPs inserted just before, on the same engine
    (each engine executes its own stream in program order, so the semantics
    are identical)."""
    import concourse.mybir as mybir

    n_nops = 0
    for f in nc.m.functions:
        for bb in f.blocks:
            insts = bb.instructions
            out = []
            changed = False
            for ins in insts:
                si = ins.sync_info
                waits = list(si.on_wait) if si is not None else []
                if len(waits) > max_waits:
                    changed = True
                    extra = waits[:-max_waits]
                    for k, w in enumerate(extra):
                        nop = mybir.InstNoOp(
                            name=f"{ins.name}-wsplit{k}", engine=ins.engine)
                        nop.sync_info = mybir.SyncInfo(
                            on_update=[], on_wait=[w])
                        out.append(nop)
                        n_nops += 1
                    while len(si.on_wait) > max_waits:
                        si.on_wait.pop(0)
                out.append(ins)
            if changed:
                bb.instructions = out
    return n_nops


def _build_nc(t_steps):
    """Build the Bass module (same NEFF for all cores; SPMD over in_maps)."""
    import contextlib

    import concourse.bass as bass
    import concourse.mybir as mybir
    import concourse.tile as tile

    dt = mybir.dt
    AF = mybir.ActivationFunctionType
    OP = mybir.AluOpType
    NT = t_steps * BL       # decoder free length

    nc = bass.Bass()
    P = nc.declare_dram_parameter

    # ---- inputs (per-core, host-prepped) ----
    wz_d = P("wz", [Z, 4 * H], dt.bfloat16, isOutput=False)        # W_ih z-part ^T
    wa_d = P("wa", [ACT + 1, 4 * H], dt.bfloat16, isOutput=False)  # [W_ih a-part | b]^T
    wh_d = P("wh", [4, 128, 4 * H], dt.bfloat16, isOutput=False)   # (0.5*W_hh)^T k-chunks
    w1_d = P("w1", [4, 128, 256], dt.bfloat16, isOutput=False)     # (0.5*enc_w1)^T
    b1_d = P("b1", [128, 2], dt.float32, isOutput=False)
    w2_d = P("w2", [2, 128, 128], dt.bfloat16, isOutput=False)     # enc_w2^T
    b2_d = P("b2", [128, 1], dt.float32, isOutput=False)
    wzz_d = P("wzz", [128, 2 * Z], dt.bfloat16, isOutput=False)    # enc_wz^T
    bzlo_d = P("bzlo", [128, 1], dt.float32, isOutput=False)
    bzhi_d = P("bzhi", [128, 1], dt.float32, isOutput=False)       # 1 + bz_hi
    dw1z_d = P("dw1z", [Z, 64], dt.bfloat16, isOutput=False)
    dw1o_d = P("dw1o", [STATE + 1, 64], dt.bfloat16, isOutput=False)  # [w1_obs | b1]^T
    dw2_d = P("dw2", [64, 64], dt.bfloat16, isOutput=False)
    db2_d = P("db2", [64, 1], dt.float32, isOutput=False)
    dw3_d = P("dw3", [64, 32], dt.bfloat16, isOutput=False)
    db3_d = P("db3", [32, 1], dt.float32, isOutput=False)
    dw4_d = P("dw4", [32, 16], dt.bfloat16, isOutput=False)
    db4_d = P("db4", [16, 1], dt.float32, isOutput=False)
    mw_d = P("mw", [16, STATE], dt.bfloat16, isOutput=False)
    mb_d = P("mb", [STATE, 1], dt.float32, isOutput=False)
    sw_d = P("sw", [16, STATE], dt.bfloat16, isOutput=False)
    sb_d = P("sb", [STATE, 1], dt.float32, isOutput=False)         # 1 + sig_b
    a_d = P("a_aug", [ACT + 1, NT], dt.bfloat16, isOutput=False)   # [a_t^T; 1]
    obs_d = P("obs_rep", [STATE + 1, 512], dt.bfloat16, isOutput=False)
    eps_d = P("eps", [t_steps, Z, BL], dt.bfloat16, isOutput=False)

    mu_o = P("mu_out", [STATE, NT], dt.float32, isOutput=True)
    s_o = P("s_out", [STATE, NT], dt.float32, isOutput=True)

    with tile.TileContext(nc) as tc, contextlib.ExitStack() as octx:
        wpool = octx.enter_context(tc.tile_pool(name="weights", bufs=1))
        spool = octx.enter_context(tc.tile_pool(name="state", bufs=1))
        gpool = octx.enter_context(tc.tile_pool(name="gwork", bufs=2))
        epool = octx.enter_context(tc.tile_pool(name="eps", bufs=4))
        dpool = octx.enter_context(tc.tile_pool(name="dwork", bufs=3))
        gpsum = octx.enter_context(
            tc.tile_pool(name="gpsum", bufs=1, space="PSUM"))
        epsum = octx.enter_context(
            tc.tile_pool(name="epsum", bufs=1, space="PSUM"))
        dpsum = octx.enter_context(
            tc.tile_pool(name="dpsum", bufs=3, space="PSUM"))

        # ---- load weights ----
        wz = wpool.tile([Z, 4 * H], dt.bfloat16)
        wa = wpool.tile([ACT + 1, 4 * H], dt.bfloat16)
        wh = wpool.tile([128, 4 * 4 * H], dt.bfloat16)
        w1 = wpool.tile([128, 4 * 256], dt.bfloat16)
        w2 = wpool.tile([128, 2 * 128], dt.bfloat16)
        wzz = wpool.tile([128, 2 * Z], dt.bfloat16)
        b1 = wpool.tile([128, 2], dt.float32)
        b2 = wpool.tile([128, 1], dt.float32)
        bzlo = wpool.tile([128, 1], dt.float32)
        bzhi = wpool.tile([128, 1], dt.float32)
        nc.sync.dma_start(out=wz[:], in_=wz_d[:])
        nc.sync.dma_start(out=wa[:], in_=wa_d[:])
        for k in range(4):
            nc.sync.dma_start(
                out=wh[:, 2048 * k:2048 * (k + 1)], in_=wh_d[k, :, :])
            nc.sync.dma_start(
                out=w1[:, 256 * k:256 * (k + 1)], in_=w1_d[k, :, :])
        for k in range(2):
            nc.sync.dma_start(
                out=w2[:, 128 * k:128 * (k + 1)], in_=w2_d[k, :, :])
        nc.sync.dma_start(out=wzz[:], in_=wzz_d[:])
        nc.sync.dma_start(out=b1[:], in_=b1_d[:])
        nc.sync.dma_start(out=b2[:], in_=b2_d[:])
        nc.sync.dma_start(out=bzlo[:], in_=bzlo_d[:])
        nc.sync.dma_start(out=bzhi[:], in_=bzhi_d[:])

        a_aug = wpool.tile([ACT + 1, NT], dt.bfloat16)
        nc.sync.dma_start(out=a_aug[:], in_=a_d[:])

        # decoder weights
        dw1z = wpool.tile([Z, 64], dt.bfloat16)
        dw1o = wpool.tile([STATE + 1, 64], dt.bfloat16)
        dw2 = wpool.tile([64, 64], dt.bfloat16)
        dw3 = wpool.tile([64, 32], dt.bfloat16)
        dw4 = wpool.tile([32, 16], dt.bfloat16)
        mw = wpool.tile([16, STATE], dt.bfloat16)
        sw = wpool.tile([16, STATE], dt.bfloat16)
        db2 = wpool.tile([64, 1], dt.float32)
        db3 = wpool.tile([32, 1], dt.float32)
        db4 = wpool.tile([16, 1], dt.float32)
        mb = wpool.tile([STATE, 1], dt.float32)
        sb = wpool.tile([STATE, 1], dt.float32)
        obs = wpool.tile([STATE + 1, 512], dt.bfloat16)
        for tdst, tsrc in [(dw1z, dw1z_d), (dw1o, dw1o_d), (dw2, dw2_d),
                           (dw3, dw3_d), (dw4, dw4_d), (mw, mw_d),
                           (sw, sw_d), (db2, db2_d), (db3, db3_d),
                           (db4, db4_d), (mb, mb_d), (sb, sb_d),
                           (obs, obs_d)]:
            nc.sync.dma_start(out=tdst[:], in_=tsrc[:])

        # ---- state ----
        h2b = spool.tile([128, H], dt.bfloat16)      # 2*h, batch-major
        h2f = spool.tile([128, H], dt.bfloat16)      # 2*h, feature-major
        cd = spool.tile([128, H], dt.float32)        # 2*c, batch-major
        z_all = spool.tile([128, NT], dt.bfloat16)   # sampled z (feature-major)
        ident = spool.tile([128, 128], dt.bfloat16)
        nc.vector.memset(h2f[:], 0.0)
        nc.vector.memset(cd[:], 0.0)
        from concourse.masks import make_identity
        make_identity(nc, ident[:])

        # gates PSUM lives across the whole loop; per-window accumulation
        # groups are (re)started by the a-part each step
        pg = gpsum.tile([128, 4 * H], dt.float32)

        def decoder_chunk(cidx):
            cs = slice(512 * cidx, 512 * (cidx + 1))
            zc = z_all[:, cs]
            p1 = dpsum.tile([64, 512], dt.float32, tag="dscr")
            nc.tensor.matmul(p1[:], dw1z[:], zc, start=True, stop=False)
            nc.tensor.matmul(p1[:], dw1o[:], obs[:], start=False, stop=True)
            d1 = dpool.tile([64, 512], dt.bfloat16, tag="d1")
            nc.vector.tensor_scalar(d1[:], p1[:], 0.0, None, OP.max)
            p2 = dpsum.tile([64, 512], dt.float32, tag="dscr")
            nc.tensor.matmul(p2[:], dw2[:], d1[:], start=True, stop=True)
            d2 = dpool.tile([64, 512], dt.bfloat16, tag="d2")
            nc.vector.tensor_scalar(d2[:], p2[:], db2[:], 0.0, OP.add, OP.max)
            p3 = dpsum.tile([32, 512], dt.float32, tag="dscr")
            nc.tensor.matmul(p3[:], dw3[:], d2[:], start=True, stop=True)
            d3 = dpool.tile([32, 512], dt.bfloat16, tag="d3")
            nc.vector.tensor_scalar(d3[:], p3[:], db3[:], 0.0, OP.add, OP.max)
            p4 = dpsum.tile([16, 512], dt.float32, tag="dscr")
            nc.tensor.matmul(p4[:], dw4[:], d3[:], start=True, stop=True)
            d4 = dpool.tile([16, 512], dt.bfloat16, tag="d4")
            nc.vector.tensor_scalar(d4[:], p4[:], db4[:], 0.0, OP.add, OP.max)
            pm = dpsum.tile([STATE, 512], dt.float32, tag="dscr")
            nc.tensor.matmul(pm[:], mw[:], d4[:], start=True, stop=True)
            mu_sb = dpool.tile([STATE, 512], dt.float32, tag="mu")
            nc.vector.tensor_scalar_add(mu_sb[:], pm[:], mb[:])
            nc.sync.dma_start(out=mu_o[:, cs], in_=mu_sb[:])
            ps = dpsum.tile([STATE, 512], dt.float32, tag="dscr")
            nc.tensor.matmul(ps[:], sw[:], d4[:], start=True, stop=True)
            s_sb = dpool.tile([STATE, 512], dt.float32, tag="sg")
            nc.vector.tensor_scalar_add(s_sb[:], ps[:], sb[:])
            nc.sync.dma_start(out=s_o[:, cs], in_=s_sb[:])

        def a_part(t, first):
            # a-part (plus folded bias): no data deps -> emitted one step
            # early so the PE fills the previous step's nonlinearity tail
            for n in range(4):
                nc.tensor.matmul(
                    pg[:, 512 * n:512 * (n + 1)],
                    a_aug[:, BL * t:BL * (t + 1)],
                    wa[:, 512 * n:512 * (n + 1)],
                    start=first, stop=False)

        # ---- recurrence (software-pipelined emission) ----
        a_part(0, True)
        for t in range(t_steps):
            eps_t = epool.tile([Z, BL], dt.bfloat16, tag="eps")
            nc.sync.dma_start(out=eps_t[:], in_=eps_d[t, :, :])

            # h-part and z-part of the gates (a-part was emitted last step)
            pairs = []
            for k in range(4):
                pairs.append((h2f[:, 128 * k:128 * (k + 1)], wh, 2048 * k))
            if t > 0:
                pairs.append((z_all[:, BL * (t - 1):BL * t], wz, 0))
            for ki, (stat, wmov, off) in enumerate(pairs):
                for n in range(4):
                    nc.tensor.matmul(
                        pg[:, 512 * n:512 * (n + 1)], stat,
                        wmov[:, off + 512 * n:off + 512 * (n + 1)],
                        start=False, stop=(ki == len(pairs) - 1))

            # tanh over all gates (i,f,o cols pre-scaled by 0.5 on host);
            # bank order f,i,g,o so the DVE chain can start early
            tg = gpool.tile([128, 4 * H], dt.float32, tag="tanh_g")
            for bank in (1, 0, 3, 2):
                bs = slice(512 * bank, 512 * (bank + 1))
                nc.scalar.activation(tg[:, bs], pg[:, bs], AF.Tanh)

            t_i = tg[:, 0:512]
            t_f = tg[:, 512:1024]
            t_o = tg[:, 1024:1536]
            t_g = tg[:, 1536:2048]

            tmp1 = gpool.tile([128, H], dt.float32, tag="tmp1")
            tmp2 = gpool.tile([128, H], dt.float32, tag="tmp2")
            # tmp1 = (1+tanh(f/2)) * cD ; tmp2 = (1+tanh(i/2)) * g
            nc.vector.scalar_tensor_tensor(
                tmp1[:], t_f, 1.0, cd[:], OP.add, OP.mult)
            nc.vector.scalar_tensor_tensor(
                tmp2[:], t_i, 1.0, t_g, OP.add, OP.mult)
            # cD = 0.5*tmp1 + tmp2   (= 2*c_new)
            nc.vector.scalar_tensor_tensor(
                cd[:], tmp1[:], 0.5, tmp2[:], OP.mult, OP.add)

            # ---- early-emitted work to fill the PE pipe during the tail
            if t + 1 < t_steps:
                a_part(t + 1, True)
            if t % 4 == 0 and t > 0:
                decoder_chunk(t // 4 - 1)

            # tc = tanh(c) = tanh(0.5*cD)
            tcn = gpool.tile([128, H], dt.float32, tag="tanh_c")
            nc.scalar.activation(tcn[:], cd[:], AF.Tanh, scale=0.5)
            # h2 = (1+tanh(o/2)) * tc  (= 2*h)
            nc.vector.scalar_tensor_tensor(
                h2b[:], t_o, 1.0, tcn[:], OP.add, OP.mult)

            # transpose h2 batch-major -> feature-major on the PE;
            # psum->sbuf copies split between ACT and DVE
            for k in range(4):
                ptr = dpsum.tile([128, 128], dt.bfloat16, tag="dscr")
                nc.tensor.transpose(
                    ptr[:], h2b[:, 128 * k:128 * (k + 1)], ident[:])
                dst = h2f[:, 128 * k:128 * (k + 1)]
                if k % 2 == 0:
                    nc.scalar.copy(dst, ptr[:])
                else:
                    nc.vector.tensor_copy(dst, ptr[:])

            # encoder (feature-major): e1/e2/zz share one PSUM bank
            pe = epsum.tile([128, 384], dt.float32, tag="enc")
            for m in range(2):
                out = pe[:, 128 * m:128 * (m + 1)]
                for k in range(4):
                    nc.tensor.matmul(
                        out, w1[:, 256 * k + 128 * m:256 * k + 128 * (m + 1)],
                        h2f[:, 128 * k:128 * (k + 1)],
                        start=(k == 0), stop=(k == 3))
            e1 = gpool.tile([128, 256], dt.bfloat16, tag="e1")
            for m in range(2):
                nc.scalar.activation(
                    e1[:, 128 * m:128 * (m + 1)],
                    pe[:, 128 * m:128 * (m + 1)],
                    AF.Relu, bias=b1[:, m:m + 1])
            # e2 = relu(W2 @ e1 + b2)  [128]
            out = pe[:, 256:384]
            for k in range(2):
                nc.tensor.matmul(
                    out, w2[:, 128 * k:128 * (k + 1)],
                    e1[:, 128 * k:128 * (k + 1)],
                    start=(k == 0), stop=(k == 1))
            e2 = gpool.tile([128, 128], dt.bfloat16, tag="e2")
            nc.scalar.activation(e2[:], out, AF.Relu, bias=b2[:])
            # zz = WZ @ e2 -> [z_loc | z_scale_pre], overlaid on pe[0:256]
            nc.tensor.matmul(pe[:, 0:Z], wzz[:, 0:Z], e2[:],
                             start=True, stop=True)
            nc.tensor.matmul(pe[:, Z:2 * Z], wzz[:, Z:2 * Z], e2[:],
                             start=True, stop=True)
            # z_scale = exp(zz_hi + (1+bz_hi))
            zsc = gpool.tile([Z, BL], dt.float32, tag="zsc")
            nc.scalar.activation(zsc[:], pe[:, Z:2 * Z], AF.Exp,
                                 bias=bzhi[:])
            zse = gpool.tile([Z, BL], dt.float32, tag="zse")
            nc.vector.tensor_tensor(zse[:], zsc[:], eps_t[:], OP.mult)
            # z = (zz_lo + bz_lo) + zsc*eps
            nc.vector.scalar_tensor_tensor(
                z_all[:, BL * t:BL * (t + 1)],
                pe[:, 0:Z], bzlo[:], zse[:], OP.add, OP.add)

        decoder_chunk(t_steps // 4 - 1)

    _split_multi_waits(nc)
    return nc


def _prep_host(inputs, t_steps):
    """Host-side weight/data prep -> per-core in_maps."""
    f32 = np.float32
    x, a = inputs["x"], inputs["a"]
    W_ih, W_hh = f32(inputs["W_ih"]), f32(inputs["W_hh"])
    b_g = f32(inputs["b_ih"]) + f32(inputs["b_hh"])

    # reorder gates (i,f,g,o) -> (i,f,o,g); scale i,f,o rows (and bias) by 0.5
    perm = np.concatenate([np.arange(0, H), np.arange(H, 2 * H),
                           np.arange(3 * H, 4 * H), np.arange(2 * H, 3 * H)])
    sc = np.ones(4 * H, f32)
    sc[:3 * H] = 0.5
    W_ih_r = W_ih[perm] * sc[:, None]
    W_hh_r = W_hh[perm] * sc[:, None]
    b_r = b_g[perm] * sc

    wz = np.ascontiguousarray(W_ih_r[:, ACT:].T).astype(bf16)        # [128, 2048]
    wa = np.ascontiguousarray(
        np.concatenate([W_ih_r[:, :ACT], b_r[:, None]], axis=1).T).astype(bf16)
    wh = np.ascontiguousarray(
        (0.5 * W_hh_r).T.reshape(4, 128, 4 * H)).astype(bf16)
    w1 = np.ascontiguousarray(
        (0.5 * f32(inputs["enc_w1"])).T.reshape(4, 128, 256)).astype(bf16)
    b1 = np.ascontiguousarray(f32(inputs["enc_b1"]).reshape(2, 128).T)
    w2 = np.ascontiguousarray(
        f32(inputs["enc_w2"]).T.reshape(2, 128, 128)).astype(bf16)
    b2 = f32(inputs["enc_b2"]).reshape(128, 1)
    wzz = np.ascontiguousarray(f32(inputs["enc_wz"]).T).astype(bf16)  # [128, 256]
    bzlo = f32(inputs["enc_bz"])[:Z].reshape(Z, 1)
    bzhi = (1.0 + f32(inputs["enc_bz"])[Z:]).reshape(Z, 1)

    dec_w1 = f32(inputs["dec_w1"])
    dw1z = np.ascontiguousarray(dec_w1[:, STATE:].T).astype(bf16)    # [128, 64]
    dw1o = np.ascontiguousarray(
        np.concatenate([dec_w1[:, :STATE], f32(inputs["dec_b1"])[:, None]],
                       axis=1).T).astype(bf16)                       # [22, 64]
    dw2 = np.ascontiguousarray(f32(inputs["dec_w2"]).T).astype(bf16)
    db2 = f32(inputs["dec_b2"]).reshape(64, 1)
    dw3 = np.ascontiguousarray(f32(inputs["dec_w3"]).T).astype(bf16)
    db3 = f32(inputs["dec_b3"]).reshape(32, 1)
    dw4 = np.ascontiguousarray(f32(inputs["dec_w4"]).T).astype(bf16)
    db4 = f32(inputs["dec_b4"]).reshape(16, 1)
    mw = np.ascontiguousarray(f32(inputs["mu_w"]).T).astype(bf16)
    mb = f32(inputs["mu_b"]).reshape(STATE, 1)
    sw = np.ascontiguousarray(f32(inputs["sig_w"]).T).astype(bf16)
    sb = (1.0 + f32(inputs["sig_b"])).reshape(STATE, 1)

    # eps via jax CPU (exact reference PRNG)
    import jax
    with jax.default_device(jax.devices("cpu")[0]):
        eps = np.asarray(jax.random.normal(
            jax.random.key(42), (T, B, Z), dtype=jax.numpy.float32))

    shared = dict(wz=wz, wa=wa, wh=wh, w1=w1, b1=b1, w2=w2, b2=b2, wzz=wzz,
                  bzlo=bzlo, bzhi=bzhi, dw1z=dw1z, dw1o=dw1o, dw2=dw2,
                  db2=db2, dw3=dw3, db3=db3, dw4=dw4, db4=db4, mw=mw, mb=mb,
                  sw=sw, sb=sb)

    in_maps = []
    ones = np.ones((1, t_steps * BL), f32)
    for ci in range(NCORES):
        bs = slice(ci * BL, (ci + 1) * BL)
        # a_aug: [9, T*BL], free index = t*BL + b
        a_c = np.ascontiguousarray(
            f32(a[bs, :t_steps, :]).transpose(2, 1, 0).reshape(
                ACT, t_steps * BL))
        a_aug = np.concatenate([a_c, ones], axis=0).astype(bf16)
        eps_c = np.ascontiguousarray(
            eps[:t_steps, bs, :].transpose(0, 2, 1)).astype(bf16)  # [T, Z, BL]
        obs_c = f32(x[bs, 0, :]).T                                 # [21, BL]
        obs_rep = np.concatenate(
            [np.tile(obs_c, (1, 512 // BL)),
             np.ones((1, 512), f32)], axis=0).astype(bf16)
        m = dict(shared)
        m.update(a_aug=a_aug, eps=eps_c, obs_rep=obs_rep)
        in_maps.append(m)
    return in_maps


def _run(inputs, t_steps=T):
    from concourse.bass_utils import run_bass_kernel_spmd

    key = ("nc", t_steps)
    if key not in _CACHE:
        _CACHE[key] = _build_nc(t_steps)
    nc = _CACHE[key]
    in_maps = _prep_host(inputs, t_steps)
    res = run_bass_kernel_spmd(nc, in_maps, list(range(NCORES)),
                               trace=False)
    return res.results


def kernel(**inputs):
    t_steps = T
    results = _run(inputs, t_steps)

    y = np.float32(inputs["y"])
    su2 = 0.0
    ss = 0.0
    sabs = 0.0
    ssd = 0.0
    n_el = NCORES * STATE * t_steps * BL
    for ci in range(NCORES):
        bs = slice(ci * BL, (ci + 1) * BL)
        mu = results[ci]["mu_out"].astype(np.float64)     # [21, T*BL]
        s = results[ci]["s_out"].astype(np.float64)       # log(sigma)
        y_c = y[bs, :t_steps, :].transpose(2, 1, 0).reshape(
            STATE, t_steps * BL).astype(np.float64)
        sd = np.exp(s)
        u = (y_c - mu) / sd
        su2 += (u * u).sum()
        ss += s.sum()
        sabs += np.abs(mu - y_c).sum()
        ssd += sd.sum()
    n_tb = NCORES * t_steps * BL
    out1 = (0.5 * su2 + ss) / n_tb + STATE * LOG_SQRT_2PI
    out2 = sabs / n_el
    out3 = ssd / n_el
    return (np.float32(out1), np.float32(out2), np.float32(out3))


if __name__ == "__main__":
    import jax
    with jax.default_device(jax.devices("cpu")[0]):
        import reference as R
        inputs = {k: np.asarray(v) for k, v in R.setup_inputs().items()}
    out = kernel(**inputs)
    print("kernel:", [float(o) for o in out])
